# revision 17
# baseline (speedup 1.0000x reference)
"""MoChA stable chunkwise attention (window w=16) on 8 Trainium2 NeuronCores.

The reference's stabilizing moving-max cancels algebraically:
    P[t] = exp(logits[t]);  S[u] = sum_{v=u-15..u} P[v]
    R[u] = emit[u]/S[u];    out[t] = P[t] * sum_{k=0..15} R[t+k]
Both width-16 window sums run on the TensorEngine as banded matmuls in a
transposed layout (partition = t mod 128, free = (block, row, chunk)); the
cross-block window wrap is handled by corner matmuls accumulating in PSUM.
Corner matmuls are merged into wide-N passes; the sequence-boundary wrap
(block 0 of S, last block of Z) uses strided rhs/out access patterns that
skip the row-start/row-end columns, so no masked scratch copies are needed.
Logits/emit travel as fp16 (exp output is fp16 anyway), the output returns
as fp16 and is upcast on the host. Junk matmuls at kernel start warm the
PE HAM clock gate (1.2 -> 2.4 GHz) while the input DMAs are in flight.

Self-contained: only numpy + concourse (on PYTHONPATH) required.
"""

import numpy as np

import concourse.bass as bass
import concourse.tile as tile
import concourse.mybir as mybir
from concourse import bacc
from concourse.bass_utils import run_bass_kernel_spmd

F32 = mybir.dt.float32
F16 = mybir.dt.float16
ACTF = mybir.ActivationFunctionType

B, T = 64, 16384
NCORES = 8
RPC = B // NCORES        # 8 rows/core
NCH = 16                 # chunks per row
NPART = 128
NBLK = 8                 # blocks per chunk
W = 16                   # window
NF = RPC * T // 128      # 1024 layout-B columns
H0 = slice(0, 512)
H1 = slice(512, 1024)

N_WARM_PRE = 8           # junk matmuls (N=512) before the S matmuls
N_WARM_MID = 0           # junk matmuls between S and Z groups


def make_consts():
    k = np.arange(128)[:, None]
    m = np.arange(128)[None, :]
    band0 = (m - k >= 0) & (m - k <= W - 1)            # S within-block
    corner = (k - m >= 128 - W + 1) & (k - m <= 127)   # S from prev block
    banda = (k - m >= 0) & (k - m <= W - 1)            # Z within-block
    cornera = (m - k >= 128 - W + 1) & (m - k <= 127)  # Z from next block
    return np.concatenate(
        [x.astype(np.float16) for x in (band0, corner, banda, cornera)],
        axis=1,
    )  # [128, 512]


def _perm(a):
    """[RPC, T] -> layout B [128, NF]: full host-side transpose, so device
    loads are plain contiguous [128 partitions x NF] DMAs (no xbar)."""
    return np.ascontiguousarray(
        a.reshape(RPC, NCH, NBLK, 128).transpose(3, 2, 0, 1).reshape(128, NF)
    )


def unperm_out(o):
    """[128, NF] layout B -> [RPC, T]."""
    return np.ascontiguousarray(
        o.reshape(128, NBLK, RPC, NCH)
        .transpose(2, 3, 1, 0)
        .reshape(RPC, T)
    )


def _wrap_out(ps_block):
    """[128,128] PSUM block view -> strided [128,8,15] skipping c=0 cols,
    offset by one chunk column."""
    return ps_block.rearrange("p (g c) -> p g c", c=16)[:, :, 1:16]


def _wrap_rhs(sb_block):
    """[128,128] SBUF block view -> strided [128,8,15] over c=0..14."""
    return sb_block.rearrange("p (g c) -> p g c", c=16)[:, :, 0:15]


def build_nc():
    nc = bacc.Bacc("TRN2", target_bir_lowering=False, debug=False,
                   num_devices=NCORES)
    lg_t = nc.dram_tensor("lg16", [NPART, NF], F16, kind="ExternalInput")
    em_t = nc.dram_tensor("em16", [NPART, NF], F16, kind="ExternalInput")
    kc_t = nc.dram_tensor("consts16", [NPART, 512], F16, kind="ExternalInput")
    out_t = nc.dram_tensor("out", [NPART, NF], F16, kind="ExternalOutput")

    with tile.TileContext(nc) as tc:
        with (
            tc.tile_pool(name="sb", bufs=1) as sb,
            tc.tile_pool(name="ps", bufs=1, space="PSUM") as ps,
        ):
            kb = sb.tile([NPART, 512], F16, tag="kb")
            warm = sb.tile([NPART, 128], F16, tag="warm")
            lg_b = sb.tile([NPART, NF], F16, tag="lg_b")
            e_b = sb.tile([NPART, NF], F16, tag="e_b")
            p_b = sb.tile([NPART, NF], F16, tag="p_b")
            r_b = sb.tile([NPART, NF], F16, tag="r_b")
            rcp_sb = sb.tile([NPART, 512], F32, tag="rcp_sb")
            o_b = sb.tile([NPART, NF], F16, tag="o_b")

            # one PSUM tile (= one bank) per half, so the tile dep tracker
            # keeps the halves' reader/writer chains independent
            s0 = ps.tile([NPART, 512], F32, tag="s0")
            s1 = ps.tile([NPART, 512], F32, tag="s1")
            z0 = ps.tile([NPART, 512], F32, tag="z0")
            z1 = ps.tile([NPART, 512], F32, tag="z1")
            w_ps = ps.tile([NPART, 512], F32, tag="w")
            # 1/S for H1 lives in PSUM: DVE mixed-dtype tensor_tensor with
            # a PSUM operand runs at 1 elem/cycle vs ~1.75 for SBUF-SBUF.
            # H0's stays in SBUF because its R-mul runs on Pool (no PSUM).
            rcp_ps = ps.tile([NPART, 512], F32, tag="rcp")

            band0 = kb[:, 0:128]
            corner = kb[:, 128:256]
            banda = kb[:, 256:384]
            cornera = kb[:, 384:512]

            # ---- warmup weight (Pool is idle early) ----
            nc.gpsimd.memset(warm[:, :], 1.0)

            # ---- loads: logits halves first (exp gates everything),
            # emit later (needed only by Rmul), consts on the ACT ring ----
            # loads: the big ones serialize FIFO on the SP ring in priority
            # order (later transfers then can't push out lg-H0's completion);
            # only the small consts load rides the ACT ring concurrently
            nc.sync.dma_start(
                lg_b[:, H0], bass.AP(lg_t, 0, [[NF, NPART], [1, 512]]))
            nc.sync.dma_start(
                lg_b[:, H1], bass.AP(lg_t, 512, [[NF, NPART], [1, 512]]))
            nc.sync.dma_start(
                e_b[:, :], bass.AP(em_t, 0, [[NF, NPART], [1, NF]]))
            nc.scalar.dma_start(
                kb[:, :], bass.AP(kc_t, 0, [[512, NPART], [1, 512]]))

            # ---- PE warmup: junk matmuls keep the HAM activity window busy
            # while DMAs land, so the real matmuls run at 2.4 GHz ----
            warm_rhs = warm[:, :].unsqueeze(1).broadcast_to([NPART, 4, 128])
            for _ in range(N_WARM_PRE):
                nc.tensor.matmul(w_ps[:, :], warm[:, :], warm_rhs,
                                 start=True, stop=True, skip_group_check=True)

            # ---- P = exp(logits), fp16; the [896:1024] slice feeds the
            # S wrap corner, so it comes right after H0 ----
            nc.scalar.activation(p_b[:, H0], lg_b[:, H0], ACTF.Exp)
            nc.scalar.activation(p_b[:, 896:1024], lg_b[:, 896:1024], ACTF.Exp)
            nc.scalar.activation(p_b[:, 512:896], lg_b[:, 512:896], ACTF.Exp)

            # ---- S matmuls ----
            nc.tensor.matmul(s0[:, :], band0, p_b[:, H0],
                             start=True, stop=False, skip_group_check=True)
            nc.tensor.matmul(s0[:, 128:512], corner, p_b[:, 0:384],
                             start=False, stop=False, skip_group_check=True)
            # sequence-wrap corner for block 0: out cols (g,c>=1) from the
            # tail of block 7 one chunk earlier; row-start cols skipped
            nc.tensor.matmul(_wrap_out(s0[:, 0:128]), corner,
                             _wrap_rhs(p_b[:, 896:1024]),
                             start=False, stop=True, skip_group_check=True)
            nc.tensor.matmul(s1[:, :], band0, p_b[:, H1],
                             start=True, stop=False, skip_group_check=True)
            nc.tensor.matmul(s1[:, :], corner, p_b[:, 384:896],
                             start=False, stop=True, skip_group_check=True)

            # ---- 1/S on DVE; R = emit * (1/S) split so the pieces gating
            # the Z matmuls land earliest: Pool takes H0, DVE takes H1 with
            # its first block (cols 512:640, needed by z0's corner) first ----
            nc.vector.reciprocal_approx_fast(rcp_sb[:, :], s0[:, :])
            nc.gpsimd.tensor_mul(r_b[:, H0], rcp_sb[:, :], e_b[:, H0])
            nc.vector.reciprocal_approx_fast(rcp_ps[:, :], s1[:, :])
            nc.vector.tensor_mul(r_b[:, 512:640], rcp_ps[:, 0:128],
                                 e_b[:, 512:640])
            nc.vector.tensor_mul(r_b[:, 640:1024], rcp_ps[:, 128:512],
                                 e_b[:, 640:1024])

            # mid warmup: junk matmuls whose rhs depends on recip H0, so
            # they fill the PE idle gap between the S and Z matmul groups
            # (keeping the HAM activity window busy) and cannot be hoisted
            # ahead of the S matmuls by the tile scheduler
            mid_rhs = rcp_sb[:, 0:256].bitcast(F16)
            for _ in range(2):
                nc.tensor.matmul(w_ps[:, :], warm[:, :], mid_rhs,
                                 start=True, stop=True, skip_group_check=True)

            # ---- Z matmuls ----
            nc.tensor.matmul(z0[:, :], banda, r_b[:, H0],
                             start=True, stop=False, skip_group_check=True)
            nc.tensor.matmul(z0[:, :], cornera, r_b[:, 128:640],
                             start=False, stop=True, skip_group_check=True)
            nc.tensor.matmul(z1[:, :], banda, r_b[:, H1],
                             start=True, stop=False, skip_group_check=True)
            nc.tensor.matmul(z1[:, 0:384], cornera, r_b[:, 640:1024],
                             start=False, stop=False, skip_group_check=True)
            # sequence-wrap corner for the last block: out cols c<=14 from
            # block 0 one chunk later; row-end cols skipped
            nc.tensor.matmul(
                z1[:, 384:512].rearrange("p (g c) -> p g c", c=16)[:, :, 0:15],
                cornera,
                r_b[:, 0:128].rearrange("p (g c) -> p g c", c=16)[:, :, 1:16],
                start=False, stop=True, skip_group_check=True)

            # ---- out = P * Z (fp16); H1 in quarters so the final
            # (tail-latency-critical) store is issued as early as possible,
            # stores alternate rings so triggers don't queue up ----
            nc.vector.tensor_mul(o_b[:, H0], p_b[:, H0], z0[:, :])
            nc.scalar.dma_start(
                bass.AP(out_t, 0, [[NF, NPART], [1, 512]]), o_b[:, H0])
            nc.vector.tensor_mul(o_b[:, H1], p_b[:, H1], z1[:, :])
            nc.sync.dma_start(
                bass.AP(out_t, 512, [[NF, NPART], [1, 512]]), o_b[:, H1])

    nc.compile()
    return nc


def make_in_maps(emit_probs, softmax_logits):
    lg16 = np.asarray(softmax_logits, dtype=np.float16)
    em16 = np.asarray(emit_probs, dtype=np.float16)
    consts = make_consts()
    maps = []
    for c in range(NCORES):
        rows = slice(c * RPC, (c + 1) * RPC)
        maps.append({
            "lg16": _perm(lg16[rows]),
            "em16": _perm(em16[rows]),
            "consts16": consts,
        })
    return maps


_NC_CACHE = None


def _get_nc():
    global _NC_CACHE
    if _NC_CACHE is None:
        _NC_CACHE = build_nc()
    return _NC_CACHE


def run(emit_probs, softmax_logits, trace=False, **kwargs):
    nc = _get_nc()
    in_maps = make_in_maps(emit_probs, softmax_logits)
    res = run_bass_kernel_spmd(
        nc, in_maps, core_ids=list(range(NCORES)), trace=trace, **kwargs
    )
    out = np.concatenate(
        [unperm_out(res.results[c]["out"]).astype(np.float32)
         for c in range(NCORES)],
        axis=0,
    )
    return out, res


def kernel(emit_probs, softmax_logits):
    return run(emit_probs, softmax_logits)[0]


# revision 18
# speedup vs baseline: 1.1757x; 1.1757x over previous
"""MoChA stable chunkwise attention (window w=16) on 8 Trainium2 NeuronCores.

The reference's stabilizing moving-max cancels algebraically:
    P[t] = exp(logits[t]);  S[u] = sum_{v=u-15..u} P[v]
    R[u] = emit[u]/S[u];    out[t] = P[t] * sum_{k=0..15} R[t+k]
Both width-16 window sums run on the TensorEngine as banded matmuls in a
transposed layout (partition = t mod 128, free = (block, row, chunk)); the
cross-block window wrap is handled by corner matmuls accumulating in PSUM.
Corner matmuls are merged into wide-N passes; the sequence-boundary wrap
(block 0 of S, last block of Z) uses strided rhs/out access patterns that
skip the row-start/row-end columns, so no masked scratch copies are needed.
Logits/emit travel as fp16 (exp output is fp16 anyway), the output returns
as fp16 and is upcast on the host. Junk matmuls at kernel start warm the
PE HAM clock gate (1.2 -> 2.4 GHz) while the input DMAs are in flight.

Self-contained: only numpy + concourse (on PYTHONPATH) required.
"""

import numpy as np

import concourse.bass as bass
import concourse.tile as tile
import concourse.mybir as mybir
from concourse import bacc
from concourse.bass_utils import run_bass_kernel_spmd

F32 = mybir.dt.float32
F16 = mybir.dt.float16
ACTF = mybir.ActivationFunctionType

B, T = 64, 16384
NCORES = 8
RPC = B // NCORES        # 8 rows/core
NCH = 16                 # chunks per row
NPART = 128
NBLK = 8                 # blocks per chunk
W = 16                   # window
NF = RPC * T // 128      # 1024 layout-B columns
H0 = slice(0, 512)
H1 = slice(512, 1024)

N_WARM_PRE = 8           # junk matmuls (N=512) before the S matmuls
N_WARM_MID = 0           # junk matmuls between S and Z groups


def make_consts():
    k = np.arange(128)[:, None]
    m = np.arange(128)[None, :]
    band0 = (m - k >= 0) & (m - k <= W - 1)            # S within-block
    corner = (k - m >= 128 - W + 1) & (k - m <= 127)   # S from prev block
    banda = (k - m >= 0) & (k - m <= W - 1)            # Z within-block
    cornera = (m - k >= 128 - W + 1) & (m - k <= 127)  # Z from next block
    return np.concatenate(
        [x.astype(np.float16) for x in (band0, corner, banda, cornera)],
        axis=1,
    )  # [128, 512]


def _perm(a):
    """[RPC, T] -> layout B [128, NF]: full host-side transpose, so device
    loads are plain contiguous [128 partitions x NF] DMAs (no xbar)."""
    return np.ascontiguousarray(
        a.reshape(RPC, NCH, NBLK, 128).transpose(3, 2, 0, 1).reshape(128, NF)
    )


def unperm_out(o):
    """[128, NF] layout B -> [RPC, T]."""
    return np.ascontiguousarray(
        o.reshape(128, NBLK, RPC, NCH)
        .transpose(2, 3, 1, 0)
        .reshape(RPC, T)
    )


def _wrap_out(ps_block):
    """[128,128] PSUM block view -> strided [128,8,15] skipping c=0 cols,
    offset by one chunk column."""
    return ps_block.rearrange("p (g c) -> p g c", c=16)[:, :, 1:16]


def _wrap_rhs(sb_block):
    """[128,128] SBUF block view -> strided [128,8,15] over c=0..14."""
    return sb_block.rearrange("p (g c) -> p g c", c=16)[:, :, 0:15]


def build_nc():
    nc = bacc.Bacc("TRN2", target_bir_lowering=False, debug=False,
                   num_devices=NCORES)
    lg_t = nc.dram_tensor("lg16", [NPART, NF], F16, kind="ExternalInput")
    em_t = nc.dram_tensor("em16", [NPART, NF], F16, kind="ExternalInput")
    kc_t = nc.dram_tensor("consts16", [NPART, 512], F16, kind="ExternalInput")
    out_t = nc.dram_tensor("out", [NPART, NF], F16, kind="ExternalOutput")

    with tile.TileContext(nc) as tc:
        with (
            tc.tile_pool(name="sb", bufs=1) as sb,
            tc.tile_pool(name="ps", bufs=1, space="PSUM") as ps,
        ):
            kb = sb.tile([NPART, 512], F16, tag="kb")
            warm = sb.tile([NPART, 128], F16, tag="warm")
            lg_b = sb.tile([NPART, NF], F16, tag="lg_b")
            e_b = sb.tile([NPART, NF], F16, tag="e_b")
            p_b = sb.tile([NPART, NF], F16, tag="p_b")
            r_b = sb.tile([NPART, NF], F16, tag="r_b")
            rcp_sb = sb.tile([NPART, 512], F32, tag="rcp_sb")
            o_b = sb.tile([NPART, NF], F16, tag="o_b")

            # one PSUM tile (= one bank) per half, so the tile dep tracker
            # keeps the halves' reader/writer chains independent
            s0 = ps.tile([NPART, 512], F32, tag="s0")
            s1 = ps.tile([NPART, 512], F32, tag="s1")
            z0 = ps.tile([NPART, 512], F32, tag="z0")
            z1 = ps.tile([NPART, 512], F32, tag="z1")
            w_ps = ps.tile([NPART, 512], F32, tag="w")
            # 1/S for H1 lives in PSUM: DVE mixed-dtype tensor_tensor with
            # a PSUM operand runs at 1 elem/cycle vs ~1.75 for SBUF-SBUF.
            # H0's stays in SBUF because its R-mul runs on Pool (no PSUM).
            rcp_ps = ps.tile([NPART, 512], F32, tag="rcp")

            band0 = kb[:, 0:128]
            corner = kb[:, 128:256]
            banda = kb[:, 256:384]
            cornera = kb[:, 384:512]

            # ---- warmup weight (Pool is idle early) ----
            nc.gpsimd.memset(warm[:, :], 1.0)

            # ---- loads: logits halves first (exp gates everything),
            # emit later (needed only by Rmul), consts on the ACT ring ----
            # loads: the big ones serialize FIFO on the SP ring in priority
            # order (later transfers then can't push out lg-H0's completion);
            # only the small consts load rides the ACT ring concurrently
            nc.sync.dma_start(
                lg_b[:, H0], bass.AP(lg_t, 0, [[NF, NPART], [1, 512]]))
            nc.sync.dma_start(
                lg_b[:, H1], bass.AP(lg_t, 512, [[NF, NPART], [1, 512]]))
            nc.sync.dma_start(
                e_b[:, :], bass.AP(em_t, 0, [[NF, NPART], [1, NF]]))
            nc.scalar.dma_start(
                kb[:, :], bass.AP(kc_t, 0, [[512, NPART], [1, 512]]))

            # ---- PE warmup: junk matmuls keep the HAM activity window busy
            # while DMAs land, so the real matmuls run at 2.4 GHz ----
            warm_rhs = warm[:, :].unsqueeze(1).broadcast_to([NPART, 4, 128])
            for _ in range(N_WARM_PRE):
                nc.tensor.matmul(w_ps[:, :], warm[:, :], warm_rhs,
                                 start=True, stop=True, skip_group_check=True)

            # ---- P = exp(logits), fp16; the [896:1024] slice feeds the
            # S wrap corner, so it comes right after H0 ----
            nc.scalar.activation(p_b[:, H0], lg_b[:, H0], ACTF.Exp)
            nc.scalar.activation(p_b[:, 896:1024], lg_b[:, 896:1024], ACTF.Exp)
            nc.scalar.activation(p_b[:, 512:896], lg_b[:, 512:896], ACTF.Exp)

            # ---- S matmuls ----
            nc.tensor.matmul(s0[:, :], band0, p_b[:, H0],
                             start=True, stop=False, skip_group_check=True)
            nc.tensor.matmul(s0[:, 128:512], corner, p_b[:, 0:384],
                             start=False, stop=False, skip_group_check=True)
            # sequence-wrap corner for block 0: out cols (g,c>=1) from the
            # tail of block 7 one chunk earlier; row-start cols skipped
            nc.tensor.matmul(_wrap_out(s0[:, 0:128]), corner,
                             _wrap_rhs(p_b[:, 896:1024]),
                             start=False, stop=True, skip_group_check=True)
            nc.tensor.matmul(s1[:, :], band0, p_b[:, H1],
                             start=True, stop=False, skip_group_check=True)
            nc.tensor.matmul(s1[:, :], corner, p_b[:, 384:896],
                             start=False, stop=True, skip_group_check=True)

            # ---- 1/S on DVE; R = emit * (1/S) split so the pieces gating
            # the Z matmuls land earliest: Pool takes H0, DVE takes H1 with
            # its first block (cols 512:640, needed by z0's corner) first ----
            nc.vector.reciprocal_approx_fast(rcp_sb[:, :], s0[:, :])
            nc.gpsimd.tensor_mul(r_b[:, H0], rcp_sb[:, :], e_b[:, H0])
            nc.vector.reciprocal_approx_fast(rcp_ps[:, :], s1[:, :])
            nc.vector.tensor_mul(r_b[:, 512:640], rcp_ps[:, 0:128],
                                 e_b[:, 512:640])
            nc.vector.tensor_mul(r_b[:, 640:1024], rcp_ps[:, 128:512],
                                 e_b[:, 640:1024])

            # mid warmup: junk matmuls whose rhs depends on recip H0, so
            # they fill the PE idle gap between the S and Z matmul groups
            # (keeping the HAM activity window busy) and cannot be hoisted
            # ahead of the S matmuls by the tile scheduler
            mid_rhs = rcp_sb[:, 0:256].bitcast(F16)
            for _ in range(3):
                nc.tensor.matmul(w_ps[:, :], warm[:, :], mid_rhs,
                                 start=True, stop=True, skip_group_check=True)

            # ---- Z matmuls ----
            nc.tensor.matmul(z0[:, :], banda, r_b[:, H0],
                             start=True, stop=False, skip_group_check=True)
            nc.tensor.matmul(z0[:, :], cornera, r_b[:, 128:640],
                             start=False, stop=True, skip_group_check=True)
            nc.tensor.matmul(z1[:, :], banda, r_b[:, H1],
                             start=True, stop=False, skip_group_check=True)
            nc.tensor.matmul(z1[:, 0:384], cornera, r_b[:, 640:1024],
                             start=False, stop=False, skip_group_check=True)
            # sequence-wrap corner for the last block: out cols c<=14 from
            # block 0 one chunk later; row-end cols skipped
            nc.tensor.matmul(
                z1[:, 384:512].rearrange("p (g c) -> p g c", c=16)[:, :, 0:15],
                cornera,
                r_b[:, 0:128].rearrange("p (g c) -> p g c", c=16)[:, :, 1:16],
                start=False, stop=True, skip_group_check=True)

            # ---- out = P * Z (fp16); H1 in quarters so the final
            # (tail-latency-critical) store is issued as early as possible,
            # stores alternate rings so triggers don't queue up ----
            nc.vector.tensor_mul(o_b[:, H0], p_b[:, H0], z0[:, :])
            nc.scalar.dma_start(
                bass.AP(out_t, 0, [[NF, NPART], [1, 512]]), o_b[:, H0])
            nc.vector.tensor_mul(o_b[:, H1], p_b[:, H1], z1[:, :])
            nc.sync.dma_start(
                bass.AP(out_t, 512, [[NF, NPART], [1, 512]]), o_b[:, H1])

    nc.compile()
    return nc


def make_in_maps(emit_probs, softmax_logits):
    lg16 = np.asarray(softmax_logits, dtype=np.float16)
    em16 = np.asarray(emit_probs, dtype=np.float16)
    consts = make_consts()
    maps = []
    for c in range(NCORES):
        rows = slice(c * RPC, (c + 1) * RPC)
        maps.append({
            "lg16": _perm(lg16[rows]),
            "em16": _perm(em16[rows]),
            "consts16": consts,
        })
    return maps


_NC_CACHE = None


def _get_nc():
    global _NC_CACHE
    if _NC_CACHE is None:
        _NC_CACHE = build_nc()
    return _NC_CACHE


def run(emit_probs, softmax_logits, trace=False, **kwargs):
    nc = _get_nc()
    in_maps = make_in_maps(emit_probs, softmax_logits)
    res = run_bass_kernel_spmd(
        nc, in_maps, core_ids=list(range(NCORES)), trace=trace, **kwargs
    )
    out = np.concatenate(
        [unperm_out(res.results[c]["out"]).astype(np.float32)
         for c in range(NCORES)],
        axis=0,
    )
    return out, res


def kernel(emit_probs, softmax_logits):
    return run(emit_probs, softmax_logits)[0]
